# revision 9
# baseline (speedup 1.0000x reference)
"""Multi-head attention (B=2, S=2048, D=1024, H=16, Hd=64) on 8 Trainium2
NeuronCores.

Sharding: 8 cores = (batch 2) x (head-quarter 4).  Core (b, hq) computes,
for batch b and heads hq*4..hq*4+3, the full-sequence partial output

    outp = (softmax-attention of its 4 heads over all 2048 q rows) @ Wo_part.T

and the host sums the four head-quarter partials per batch and adds bo.
No K/V projection is duplicated (unlike a q-split layout), so phase-1
tensor work is exactly 1/8 of the global total per core.

Everything is bf16 on the wire and in SBUF (PSUM accumulates fp32):
  xC     [4, 128, 8, 512]  x[b].T chunked so each 512-q-column chunk is
                           per-partition contiguous (big DMA descriptors)
  wqT/wkT/wvT [D, 256]  W.T column slice
  woT    [256, D]  Wo.T row slice
  maskT  [S, S]    keep-mask (mask[b,0]==0).T  bf16 0/1, 8MB
  outp   [S, D]    partial output bf16 (host sums + bo in fp32)

Head packing: 4 heads = 2 head-pairs; a pair's two heads live on
partition halves 0-63 / 64-127 so the pair's two scores matmuls run
concurrently on disjoint PE row groups.

Pipeline per core:
  1. projections in 8-k-tile PSUM chains, one 512-column x chunk at a
     time (chains start as soon as chunk 0 lands).  V lands in v_aug
     [128 s, 16 sb, head*128 + (64 V | 64 ones)]; the ones columns make
     the attnV matmul accumulate Z = sum(expm) into PSUM rows 64..127.
  2. per (pair c, q-chunk j), 16 s_k tiles i:  scT pair -> exp (ScalarE,
     the ~133us critical engine) -> keep-mask multiply (VectorE) ->
     attnV accumulate.  Leftover phase-1 chains and phase-3 blocks are
     sprinkled one sub-microsecond piece per i-slot to keep the PE dense
     without stalling the exp stream.
  3. per q-chunk j: out[q, D] = out_cT.T @ woT into PSUM, evicted bf16,
     DMA out.  Softmax normalization (reciprocal of Z, broadcast,
     multiply) happens between phases 2 and 3 per (c, j).
"""

import sys

if "/opt/trn_rl_repo" not in sys.path:
    sys.path.insert(0, "/opt/trn_rl_repo")

import numpy as np

B, S, D = 2, 2048, 1024
H, HD = 16, 64
NCORES = 8
HPC = 4  # heads per core
DPC = HPC * HD  # 256 head dims per core
KT = D // 128  # 8 contraction tiles
NSK = S // 128  # 16 s_k tiles
NJ = S // 512  # 4 q chunks
NC2 = HPC // 2  # 2 head pairs

_CACHE = {}


def _build():
    import concourse.bacc as bacc
    import concourse.mybir as mybir
    import concourse.tile as tile

    F32 = mybir.dt.float32
    BF16 = mybir.dt.bfloat16
    MULT = mybir.AluOpType.mult
    EXP = mybir.ActivationFunctionType.Exp

    nc = bacc.Bacc("TRN2", target_bir_lowering=False, debug=False)

    xC = nc.dram_tensor("xC", [NJ, 128, KT, 512], BF16, kind="ExternalInput")
    wqT = nc.dram_tensor("wqT", [D, DPC], BF16, kind="ExternalInput")
    wkT = nc.dram_tensor("wkT", [D, DPC], BF16, kind="ExternalInput")
    wvT = nc.dram_tensor("wvT", [D, DPC], BF16, kind="ExternalInput")
    woT = nc.dram_tensor("woT", [DPC, D], BF16, kind="ExternalInput")
    maskT = nc.dram_tensor("maskT", [S, S], BF16, kind="ExternalInput")
    outp = nc.dram_tensor("outp", [S, D], BF16, kind="ExternalOutput")

    xC_r = xC.rearrange("j p t u -> p j t u")  # [128, NJ, KT, 512]
    wqT_r = wqT.rearrange("(t p) d -> p t d", p=128)  # [128, KT, DPC]
    wkT_r = wkT.rearrange("(t p) d -> p t d", p=128)
    wvT_r = wvT.rearrange("(t p) d -> p t d", p=128)
    woT_r = woT.rearrange("(c p) d -> p c d", p=128)  # [128, 2, D]
    maskT_r = maskT.rearrange("(i p) q -> p i q", p=128)  # [128, NSK, S]

    with tile.TileContext(nc) as tc:
        with (
            tc.tile_pool(name="keep", bufs=1) as keep,
            tc.tile_pool(name="pexpt", bufs=5) as pexpt,
            tc.tile_pool(name="pexpm", bufs=6) as pexpm,
            tc.tile_pool(name="pnorm", bufs=2) as pnorm,
            tc.tile_pool(name="p3s", bufs=3) as p3s,
            tc.tile_pool(name="scp", bufs=2, space="PSUM") as scp,
            tc.tile_pool(name="opp", bufs=1, space="PSUM") as opp,
            tc.tile_pool(name="aux", bufs=2, space="PSUM") as aux,
        ):
            # ---- persistent SBUF ----------------------------------------
            x_sb = keep.tile([128, KT, S], BF16)  # 32KB/part
            wq_sb = keep.tile([128, KT, DPC], BF16)
            wk_sb = keep.tile([128, KT, DPC], BF16)
            wv_sb = keep.tile([128, KT, DPC], BF16)
            wo_sb = keep.tile([128, 2, D], BF16)
            qT_sb = keep.tile([128, NC2, S], BF16)
            kT_sb = keep.tile([128, NC2, S], BF16)
            v_aug = keep.tile([128, NSK, HPC * 128], BF16)  # 16KB/part
            mask01 = keep.tile([128, NSK, S], BF16)  # 64KB/part
            out_cT = keep.tile([128, NC2, S], BF16)

            nc.any.memset(v_aug[:], 1.0)

            # ---- DMAs, all on the SP HWDGE queue in need-order ----------
            # (bulk data must avoid the gpsimd software DGE: its software
            # descriptor generation trickles ~8MB over the whole kernel)
            nc.sync.dma_start(out=wk_sb[:], in_=wkT_r[:])
            nc.sync.dma_start(out=wq_sb[:], in_=wqT_r[:])

            def dma_x(jc):
                sl = slice(jc * 512, (jc + 1) * 512)
                nc.sync.dma_start(out=x_sb[:, :, sl], in_=xC_r[:, jc, :, :])

            def dma_mask(i, half):
                sl = slice(half * 1024, (half + 1) * 1024)
                nc.sync.dma_start(
                    out=mask01[:, i, sl], in_=maskT_r[:, i, sl]
                )

            dma_x(0)
            nc.sync.dma_start(out=wv_sb[:], in_=wvT_r[:])
            for i in range(4):
                dma_mask(i, 0)
            dma_x(1)
            for i in range(4, 8):
                dma_mask(i, 0)
            dma_x(2)
            for i in range(8, 12):
                dma_mask(i, 0)
            dma_x(3)
            for i in range(12, 16):
                dma_mask(i, 0)
            for i in range(16):
                dma_mask(i, 1)
            nc.sync.dma_start(out=wo_sb[:], in_=woT_r[:])

            # ---- phase-1 chains, split into two sub-us halves -----------
            # evict_eng: 'scalar' for the prefix (ACT idle before the exp
            # stream ramps), 'vector' afterwards.
            def chain_kq(w_sb, dst_sb, c, jk, evict_eng="vector"):
                ps = aux.tile([128, 512], F32, tag="aux", name=f"ch_{id(w_sb)}_{c}_{jk}")
                sl = slice(jk * 512, (jk + 1) * 512)

                def half(r0, r1):
                    for t in range(r0, r1):
                        nc.tensor.matmul(
                            ps[:],
                            w_sb[:, t, c * 128 : (c + 1) * 128],
                            x_sb[:, t, sl],
                            start=(t == 0),
                            stop=(t == KT - 1),
                        )
                    if r1 == KT:
                        eng = nc.scalar if evict_eng == "scalar" else nc.vector
                        if evict_eng == "scalar":
                            eng.copy(dst_sb[:, c, sl], ps[:])
                        else:
                            eng.tensor_copy(dst_sb[:, c, sl], ps[:])

                return [lambda: half(0, 4), lambda: half(4, KT)]

            def chain_v(sb):
                ps = aux.tile([128, 256], F32, tag="aux", name=f"chv_{sb}")

                def half(r0, r1):
                    for t in range(r0, r1):
                        nc.tensor.matmul(
                            ps[:],
                            x_sb[:, t, sb * 128 : (sb + 1) * 128],
                            wv_sb[:, t, :],
                            start=(t == 0),
                            stop=(t == KT - 1),
                        )
                    if r1 == KT:
                        nc.vector.tensor_copy(
                            v_aug[:, sb, :]
                            .rearrange("p (h c2) -> p h c2", h=HPC)[:, :, 0:HD],
                            ps[:].rearrange("p (h c2) -> p h c2", h=HPC),
                        )

                return [lambda: half(0, 4), lambda: half(4, KT)]

            def phase3_block(mm, n):
                msl = slice(mm * 128, (mm + 1) * 128)
                ps = aux.tile([128, 512], F32, tag="aux", name=f"p3_{mm}_{n}")
                for cb in range(2):
                    nc.tensor.matmul(
                        ps[:],
                        out_cT[:, cb, msl],
                        wo_sb[:, cb, n * 512 : (n + 1) * 512],
                        start=(cb == 0),
                        stop=(cb == 1),
                    )
                ob = p3s.tile([128, 512], BF16, tag="ob")
                nc.any.tensor_copy(ob[:], ps[:])
                nc.sync.dma_start(
                    out=outp[msl, n * 512 : (n + 1) * 512], in_=ob[:]
                )

            def phase3_thunks(j):
                return [
                    (lambda mm=j * 4 + m, n=n: phase3_block(mm, n))
                    for m in range(4)
                    for n in range(2)
                ]

            # ---- phase-2 unit -------------------------------------------
            LOOKAHEAD = 3

            def unit(c, j, extras=()):
                jsl = slice(j * 512, (j + 1) * 512)
                out_ps = opp.tile([128, 2, 512], F32, tag="ops")
                expm_q = {}
                extras = list(extras)
                for ii in range(NSK + LOOKAHEAD):
                    if extras:
                        extras.pop(0)()
                    if ii < NSK:
                        i = ii
                        sc = scp.tile([128, 2, 512], F32, tag="sc")
                        for h2 in range(2):
                            hsl = slice(h2 * 64, (h2 + 1) * 64)
                            nc.tensor.matmul(
                                sc[:, h2, :],
                                kT_sb[hsl, c, i * 128 : (i + 1) * 128],
                                qT_sb[hsl, c, jsl],
                                start=True,
                                stop=True,
                            )
                        expt = pexpt.tile([128, 2, 512], BF16, tag="expt")
                        nc.scalar.activation(
                            out=expt[:], in_=sc[:], func=EXP, scale=0.125
                        )
                        expm = pexpm.tile(
                            [128, 2, 512], BF16, tag="expm",
                            name=f"expm_{c}_{j}_{i}",
                        )
                        for h2 in range(2):
                            nc.vector.tensor_tensor(
                                out=expm[:, h2, :],
                                in0=expt[:, h2, :],
                                in1=mask01[:, i, jsl],
                                op=MULT,
                            )
                        expm_q[i] = expm
                    if ii >= LOOKAHEAD:
                        i = ii - LOOKAHEAD
                        expm = expm_q.pop(i)
                        for h2 in range(2):
                            h = 2 * c + h2
                            nc.tensor.matmul(
                                out_ps[:, h2, :],
                                v_aug[:, i, h * 128 : (h + 1) * 128],
                                expm[:, h2, :],
                                start=(i == 0),
                                stop=(i == NSK - 1),
                            )
                # normalize: PSUM rows 64..127 hold Z replicated.  Copy one
                # Z row to partition 0, reciprocal there, broadcast on
                # gpsimd, multiply (HW-proven sequence; partition-shifted
                # custom-DVE ops diverge from CoreSim on hardware).
                for h2 in range(2):
                    zrow = pnorm.tile([1, 512], F32, tag="zrow")
                    nc.vector.tensor_copy(zrow[:], out_ps[64:65, h2, :])
                    zr1 = pnorm.tile([1, 512], F32, tag="zr1")
                    nc.vector.reciprocal_approx_fast(out=zr1[:], in_=zrow[:])
                    zr = pnorm.tile([64, 512], F32, tag="zr")
                    nc.gpsimd.partition_broadcast(zr[:], zr1[:])
                    nc.vector.tensor_tensor(
                        out=out_cT[h2 * 64 : (h2 + 1) * 64, c, jsl],
                        in0=out_ps[0:64, h2, :],
                        in1=zr[:],
                        op=MULT,
                    )
                for t in extras:
                    t()

            # ---- emission in intended execution order -------------------
            def K(c, jk, ev="vector"):
                return chain_kq(wk_sb, kT_sb, c, jk, ev)

            def Q(c, jq, ev="vector"):
                return chain_kq(wq_sb, qT_sb, c, jq, ev)

            # prefix: everything unit (c0, j0) needs, paced by x chunks
            prefix = (
                K(0, 0, "scalar") + Q(0, 0, "scalar") + chain_v(0) + chain_v(1)
                + chain_v(2) + chain_v(3)
                + K(0, 1, "scalar") + chain_v(4) + chain_v(5) + chain_v(6)
                + chain_v(7)
                + K(0, 2, "scalar") + chain_v(8) + chain_v(9) + chain_v(10)
                + chain_v(11)
                + K(0, 3, "scalar") + chain_v(12) + chain_v(13) + chain_v(14)
                + chain_v(15)
            )
            for t in prefix:
                t()

            unit(0, 0, extras=K(1, 0) + K(1, 1) + K(1, 2) + K(1, 3) + Q(1, 0))
            unit(1, 0, extras=Q(0, 1) + Q(1, 1))
            unit(0, 1, extras=Q(0, 2) + Q(1, 2))
            unit(1, 1, extras=Q(0, 3) + Q(1, 3))
            unit(0, 2, extras=phase3_thunks(0))
            unit(1, 2, extras=phase3_thunks(1))
            unit(0, 3, extras=phase3_thunks(2))
            unit(1, 3)
            for t in phase3_thunks(3):
                t()

    nc.compile()
    return nc


def _get_nc():
    if "nc" not in _CACHE:
        _CACHE["nc"] = _build()
    return _CACHE["nc"]


def _prep_inputs(x, mask, Wq, Wk, Wv, Wo, bo):
    """Build the 8 per-core input maps (bf16 on the wire)."""
    import ml_dtypes

    bf16 = ml_dtypes.bfloat16
    x = np.asarray(x, dtype=np.float32)
    mask = np.asarray(mask, dtype=np.int32)
    wqT = np.asarray(Wq, np.float32).T
    wkT = np.asarray(Wk, np.float32).T
    wvT = np.asarray(Wv, np.float32).T
    woT = np.asarray(Wo, np.float32).T

    # x[b].T chunked: [NJ, 128, KT, 512] with xC[j, p, t, u] =
    # x[b].T[t*128+p, j*512+u]  (per-partition contiguous chunk DMAs)
    xCs = []
    for b in range(B):
        xT = x[b].T.astype(bf16)  # [D, S]
        xc = np.ascontiguousarray(
            xT.reshape(KT, 128, NJ, 512).transpose(2, 1, 0, 3)
        )
        xCs.append(xc)
    maskTs = [
        (np.ascontiguousarray(mask[b, 0].T) == 0).astype(bf16) for b in range(B)
    ]

    in_maps = []
    for c in range(NCORES):
        b, hq = c >> 2, c & 3
        doff = hq * DPC
        in_maps.append(
            {
                "xC": xCs[b],
                "wqT": np.ascontiguousarray(wqT[:, doff : doff + DPC]).astype(bf16),
                "wkT": np.ascontiguousarray(wkT[:, doff : doff + DPC]).astype(bf16),
                "wvT": np.ascontiguousarray(wvT[:, doff : doff + DPC]).astype(bf16),
                "woT": np.ascontiguousarray(woT[doff : doff + DPC, :]).astype(bf16),
                "maskT": maskTs[b],
            }
        )
    return in_maps


def run(inputs: dict, trace: bool = False):
    """Run the kernel; returns (full_output, BassKernelResults)."""
    from concourse.bass_utils import run_bass_kernel_spmd

    nc = _get_nc()
    in_maps = _prep_inputs(**inputs)
    res = run_bass_kernel_spmd(
        nc, in_maps, core_ids=list(range(NCORES)), trace=trace
    )
    bo = np.asarray(inputs["bo"], dtype=np.float32)
    out = np.empty((B, S, D), dtype=np.float32)
    for b in range(B):
        acc = res.results[4 * b]["outp"].astype(np.float32)
        for hq in range(1, 4):
            acc = acc + res.results[4 * b + hq]["outp"].astype(np.float32)
        out[b] = acc + bo[None, :]
    return out, res


def kernel(**inputs) -> np.ndarray:
    out, _ = run(inputs, trace=False)
    return out


# revision 14
# speedup vs baseline: 1.1478x; 1.1478x over previous
"""Multi-head attention (B=2, S=2048, D=1024, H=16, Hd=64) on 8 Trainium2
NeuronCores.

Sharding: 8 cores = (batch 2) x (head-quarter 4).  Core (b, hq) computes,
for batch b and heads hq*4..hq*4+3, the full-sequence partial output

    outp = (softmax-attention of its 4 heads over all 2048 q rows) @ Wo_part.T

and the host sums the four head-quarter partials per batch and adds bo.
No K/V projection is duplicated (unlike a q-split layout), so phase-1
tensor work is exactly 1/8 of the global total per core.

Everything is bf16 on the wire and in SBUF (PSUM accumulates fp32):
  xC     [4, 128, 8, 512]  x[b].T chunked so each 512-q-column chunk is
                           per-partition contiguous (big DMA descriptors)
  wqT/wkT/wvT [D, 256]  W.T column slice
  woT    [256, D]  Wo.T row slice
  maskT  [S, S]    keep-mask (mask[b,0]==0).T  bf16 0/1, 8MB
  outp   [S, D]    partial output bf16 (host sums + bo in fp32)

Head packing: 4 heads = 2 head-pairs; a pair's two heads live on
partition halves 0-63 / 64-127 so the pair's two scores matmuls run
concurrently on disjoint PE row groups.

Pipeline per core:
  1. projections in 8-k-tile PSUM chains, one 512-column x chunk at a
     time (chains start as soon as chunk 0 lands).  V lands in v_aug
     [128 s, 16 sb, head*128 + (64 V | 64 ones)]; the ones columns make
     the attnV matmul accumulate Z = sum(expm) into PSUM rows 64..127.
  2. per (pair c, q-chunk j), 16 s_k tiles i:  scT pair -> exp (ScalarE,
     the ~133us critical engine) -> keep-mask multiply (VectorE) ->
     attnV accumulate.  Leftover phase-1 chains and phase-3 blocks are
     sprinkled one sub-microsecond piece per i-slot to keep the PE dense
     without stalling the exp stream.
  3. per q-chunk j: out[q, D] = out_cT.T @ woT into PSUM, evicted bf16,
     DMA out.  Softmax normalization (reciprocal of Z, broadcast,
     multiply) happens between phases 2 and 3 per (c, j).
"""

import sys

if "/opt/trn_rl_repo" not in sys.path:
    sys.path.insert(0, "/opt/trn_rl_repo")

import numpy as np

B, S, D = 2, 2048, 1024
H, HD = 16, 64
NCORES = 8
HPC = 4  # heads per core
DPC = HPC * HD  # 256 head dims per core
KT = D // 128  # 8 contraction tiles
NSK = S // 128  # 16 s_k tiles
NJ = S // 512  # 4 q chunks
NC2 = HPC // 2  # 2 head pairs

_CACHE = {}


def _build():
    import concourse.bacc as bacc
    import concourse.mybir as mybir
    import concourse.tile as tile

    F32 = mybir.dt.float32
    BF16 = mybir.dt.bfloat16
    MULT = mybir.AluOpType.mult
    EXP = mybir.ActivationFunctionType.Exp

    nc = bacc.Bacc("TRN2", target_bir_lowering=False, debug=False)

    xC = nc.dram_tensor("xC", [NJ, 128, KT, 512], BF16, kind="ExternalInput")
    wqT = nc.dram_tensor("wqT", [D, DPC], BF16, kind="ExternalInput")
    wkT = nc.dram_tensor("wkT", [D, DPC], BF16, kind="ExternalInput")
    wvT = nc.dram_tensor("wvT", [D, DPC], BF16, kind="ExternalInput")
    woT = nc.dram_tensor("woT", [DPC, D], BF16, kind="ExternalInput")
    maskT = nc.dram_tensor("maskT", [S, S], BF16, kind="ExternalInput")
    outp = nc.dram_tensor("outp", [S, D], BF16, kind="ExternalOutput")

    xC_r = xC.rearrange("j p t u -> p j t u")  # [128, NJ, KT, 512]
    wqT_r = wqT.rearrange("(t p) d -> p t d", p=128)  # [128, KT, DPC]
    wkT_r = wkT.rearrange("(t p) d -> p t d", p=128)
    wvT_r = wvT.rearrange("(t p) d -> p t d", p=128)
    woT_r = woT.rearrange("(c p) d -> p c d", p=128)  # [128, 2, D]
    maskT_r = maskT.rearrange("(i p) q -> p i q", p=128)  # [128, NSK, S]

    with tile.TileContext(nc) as tc:
        with (
            tc.tile_pool(name="keep", bufs=1) as keep,
            tc.tile_pool(name="pexpt", bufs=6) as pexpt,
            tc.tile_pool(name="pexpm", bufs=7) as pexpm,
            tc.tile_pool(name="pnorm", bufs=2) as pnorm,
            tc.tile_pool(name="p3s", bufs=3) as p3s,
            tc.tile_pool(name="scp", bufs=2, space="PSUM") as scp,
            tc.tile_pool(name="opp", bufs=1, space="PSUM") as opp,
            tc.tile_pool(name="aux", bufs=2, space="PSUM") as aux,
        ):
            # ---- persistent SBUF ----------------------------------------
            x_sb = keep.tile([128, KT, S], BF16)  # 32KB/part
            wq_sb = keep.tile([128, KT, DPC], BF16)
            wk_sb = keep.tile([128, KT, DPC], BF16)
            wv_sb = keep.tile([128, KT, DPC], BF16)
            wo_sb = keep.tile([128, 2, D], BF16)
            qT_sb = keep.tile([128, NC2, S], BF16)
            kT_sb = keep.tile([128, NC2, S], BF16)
            v_aug = keep.tile([128, NSK, HPC * 128], BF16)  # 16KB/part
            mask01 = keep.tile([128, NSK, S], BF16)  # 64KB/part
            out_cT = keep.tile([128, NC2, S], BF16)

            nc.any.memset(v_aug[:], 1.0)

            # ---- DMAs, all on the SP HWDGE queue in need-order ----------
            # (bulk data must avoid the gpsimd software DGE: its software
            # descriptor generation trickles ~8MB over the whole kernel)
            def dma_x(jc):
                sl = slice(jc * 512, (jc + 1) * 512)
                nc.sync.dma_start(out=x_sb[:, :, sl], in_=xC_r[:, jc, :, :])

            def dma_mask(i):
                nc.sync.dma_start(out=mask01[:, i, :], in_=maskT_r[:, i, :])

            dma_x(0)
            nc.sync.dma_start(out=wk_sb[:], in_=wkT_r[:])
            nc.sync.dma_start(out=wq_sb[:], in_=wqT_r[:])
            nc.sync.dma_start(out=wv_sb[:], in_=wvT_r[:])
            for i in range(4):
                dma_mask(i)
            dma_x(1)
            for i in range(4, 8):
                dma_mask(i)
            dma_x(2)
            for i in range(8, 12):
                dma_mask(i)
            dma_x(3)
            for i in range(12, 16):
                dma_mask(i)
            nc.sync.dma_start(out=wo_sb[:], in_=woT_r[:])

            # ---- phase-1 chains, split into two sub-us halves -----------
            # evict_eng: 'scalar' for the prefix (ACT idle before the exp
            # stream ramps), 'vector' afterwards.
            def chain_kq(w_sb, dst_sb, c, jk, evict_eng="vector"):
                ps = aux.tile([128, 512], F32, tag="aux", name=f"ch_{id(w_sb)}_{c}_{jk}")
                sl = slice(jk * 512, (jk + 1) * 512)

                def half(r0, r1):
                    for t in range(r0, r1):
                        nc.tensor.matmul(
                            ps[:],
                            w_sb[:, t, c * 128 : (c + 1) * 128],
                            x_sb[:, t, sl],
                            start=(t == 0),
                            stop=(t == KT - 1),
                        )
                    if r1 == KT:
                        eng = nc.scalar if evict_eng == "scalar" else nc.vector
                        if evict_eng == "scalar":
                            eng.copy(dst_sb[:, c, sl], ps[:])
                        else:
                            eng.tensor_copy(dst_sb[:, c, sl], ps[:])

                return [lambda: half(0, 4), lambda: half(4, KT)]

            def chain_v(sb):
                ps = aux.tile([128, 256], F32, tag="aux", name=f"chv_{sb}")

                def half(r0, r1):
                    for t in range(r0, r1):
                        nc.tensor.matmul(
                            ps[:],
                            x_sb[:, t, sb * 128 : (sb + 1) * 128],
                            wv_sb[:, t, :],
                            start=(t == 0),
                            stop=(t == KT - 1),
                        )
                    if r1 == KT:
                        nc.vector.tensor_copy(
                            v_aug[:, sb, :]
                            .rearrange("p (h c2) -> p h c2", h=HPC)[:, :, 0:HD],
                            ps[:].rearrange("p (h c2) -> p h c2", h=HPC),
                        )

                return [lambda: half(0, 4), lambda: half(4, KT)]

            def phase3_block(mm, n):
                msl = slice(mm * 128, (mm + 1) * 128)
                ps = aux.tile([128, 512], F32, tag="aux", name=f"p3_{mm}_{n}")
                for cb in range(2):
                    nc.tensor.matmul(
                        ps[:],
                        out_cT[:, cb, msl],
                        wo_sb[:, cb, n * 512 : (n + 1) * 512],
                        start=(cb == 0),
                        stop=(cb == 1),
                    )
                ob = p3s.tile([128, 512], BF16, tag="ob")
                nc.any.tensor_copy(ob[:], ps[:])
                nc.sync.dma_start(
                    out=outp[msl, n * 512 : (n + 1) * 512], in_=ob[:]
                )

            def phase3_thunks(j):
                return [
                    (lambda mm=j * 4 + m, n=n: phase3_block(mm, n))
                    for m in range(4)
                    for n in range(2)
                ]

            # ---- phase-2 unit -------------------------------------------
            LOOKAHEAD = 5

            def unit(c, j, extras=()):
                jsl = slice(j * 512, (j + 1) * 512)
                out_ps = opp.tile([128, 2, 512], F32, tag="ops")
                expm_q = {}
                extras = list(extras)
                for ii in range(NSK + LOOKAHEAD):
                    if extras:
                        extras.pop(0)()
                    if ii < NSK:
                        i = ii
                        sc = scp.tile([128, 2, 512], F32, tag="sc")
                        for h2 in range(2):
                            hsl = slice(h2 * 64, (h2 + 1) * 64)
                            nc.tensor.matmul(
                                sc[:, h2, :],
                                kT_sb[hsl, c, i * 128 : (i + 1) * 128],
                                qT_sb[hsl, c, jsl],
                                start=True,
                                stop=True,
                            )
                        expt = pexpt.tile([128, 2, 512], BF16, tag="expt")
                        nc.scalar.activation(
                            out=expt[:], in_=sc[:], func=EXP, scale=0.125
                        )
                        expm = pexpm.tile(
                            [128, 2, 512], BF16, tag="expm",
                            name=f"expm_{c}_{j}_{i}",
                        )
                        nc.vector.tensor_tensor(
                            out=expm[:],
                            in0=expt[:],
                            in1=mask01[:, i, jsl][:, None, :].to_broadcast(
                                (128, 2, 512)
                            ),
                            op=MULT,
                        )
                        expm_q[i] = expm
                    if ii >= LOOKAHEAD:
                        i = ii - LOOKAHEAD
                        expm = expm_q.pop(i)
                        for h2 in range(2):
                            h = 2 * c + h2
                            nc.tensor.matmul(
                                out_ps[:, h2, :],
                                v_aug[:, i, h * 128 : (h + 1) * 128],
                                expm[:, h2, :],
                                start=(i == 0),
                                stop=(i == NSK - 1),
                            )
                # normalize: PSUM rows 64..127 hold Z replicated.  Copy one
                # Z row to partition 0, reciprocal there, broadcast on
                # gpsimd, multiply (HW-proven sequence; partition-shifted
                # custom-DVE ops diverge from CoreSim on hardware).
                for h2 in range(2):
                    zrow = pnorm.tile([1, 512], F32, tag="zrow")
                    nc.vector.tensor_copy(zrow[:], out_ps[64:65, h2, :])
                    zr1 = pnorm.tile([1, 512], F32, tag="zr1")
                    nc.vector.reciprocal_approx_fast(out=zr1[:], in_=zrow[:])
                    zr = pnorm.tile([64, 512], F32, tag="zr")
                    nc.gpsimd.partition_broadcast(zr[:], zr1[:])
                    nc.vector.tensor_tensor(
                        out=out_cT[h2 * 64 : (h2 + 1) * 64, c, jsl],
                        in0=out_ps[0:64, h2, :],
                        in1=zr[:],
                        op=MULT,
                    )
                for t in extras:
                    t()

            # ---- emission in intended execution order -------------------
            def K(c, jk, ev="vector"):
                return chain_kq(wk_sb, kT_sb, c, jk, ev)

            def Q(c, jq, ev="vector"):
                return chain_kq(wq_sb, qT_sb, c, jq, ev)

            # prefix: everything unit (c0, j0) needs, paced by x chunks
            prefix = (
                K(0, 0) + Q(0, 0) + chain_v(0) + chain_v(1)
                + chain_v(2) + chain_v(3)
                + K(0, 1) + chain_v(4) + chain_v(5) + chain_v(6)
                + chain_v(7)
                + K(0, 2) + chain_v(8) + chain_v(9) + chain_v(10)
                + chain_v(11)
                + K(0, 3) + chain_v(12) + chain_v(13) + chain_v(14)
                + chain_v(15)
            )
            for t in prefix:
                t()

            unit(0, 0, extras=K(1, 0) + K(1, 1) + K(1, 2) + K(1, 3) + Q(1, 0))
            unit(1, 0, extras=Q(0, 1) + Q(1, 1))
            unit(0, 1, extras=Q(0, 2) + Q(1, 2))
            unit(1, 1, extras=Q(0, 3) + Q(1, 3))
            unit(0, 2, extras=phase3_thunks(0))
            unit(1, 2, extras=phase3_thunks(1))
            unit(0, 3, extras=phase3_thunks(2))
            unit(1, 3)
            for t in phase3_thunks(3):
                t()

    nc.compile()
    return nc


def _get_nc():
    if "nc" not in _CACHE:
        _CACHE["nc"] = _build()
    return _CACHE["nc"]


def _prep_inputs(x, mask, Wq, Wk, Wv, Wo, bo):
    """Build the 8 per-core input maps (bf16 on the wire)."""
    import ml_dtypes

    bf16 = ml_dtypes.bfloat16
    x = np.asarray(x, dtype=np.float32)
    mask = np.asarray(mask, dtype=np.int32)
    wqT = np.asarray(Wq, np.float32).T
    wkT = np.asarray(Wk, np.float32).T
    wvT = np.asarray(Wv, np.float32).T
    woT = np.asarray(Wo, np.float32).T

    # x[b].T chunked: [NJ, 128, KT, 512] with xC[j, p, t, u] =
    # x[b].T[t*128+p, j*512+u]  (per-partition contiguous chunk DMAs)
    xCs = []
    for b in range(B):
        xT = x[b].T.astype(bf16)  # [D, S]
        xc = np.ascontiguousarray(
            xT.reshape(KT, 128, NJ, 512).transpose(2, 1, 0, 3)
        )
        xCs.append(xc)
    maskTs = [
        (np.ascontiguousarray(mask[b, 0].T) == 0).astype(bf16) for b in range(B)
    ]

    in_maps = []
    for c in range(NCORES):
        b, hq = c >> 2, c & 3
        doff = hq * DPC
        in_maps.append(
            {
                "xC": xCs[b],
                "wqT": np.ascontiguousarray(wqT[:, doff : doff + DPC]).astype(bf16),
                "wkT": np.ascontiguousarray(wkT[:, doff : doff + DPC]).astype(bf16),
                "wvT": np.ascontiguousarray(wvT[:, doff : doff + DPC]).astype(bf16),
                "woT": np.ascontiguousarray(woT[doff : doff + DPC, :]).astype(bf16),
                "maskT": maskTs[b],
            }
        )
    return in_maps


def run(inputs: dict, trace: bool = False):
    """Run the kernel; returns (full_output, BassKernelResults)."""
    from concourse.bass_utils import run_bass_kernel_spmd

    nc = _get_nc()
    in_maps = _prep_inputs(**inputs)
    res = run_bass_kernel_spmd(
        nc, in_maps, core_ids=list(range(NCORES)), trace=trace
    )
    bo = np.asarray(inputs["bo"], dtype=np.float32)
    out = np.empty((B, S, D), dtype=np.float32)
    for b in range(B):
        acc = res.results[4 * b]["outp"].astype(np.float32)
        for hq in range(1, 4):
            acc = acc + res.results[4 * b + hq]["outp"].astype(np.float32)
        out[b] = acc + bo[None, :]
    return out, res


def kernel(**inputs) -> np.ndarray:
    out, _ = run(inputs, trace=False)
    return out
